# revision 33
# baseline (speedup 1.0000x reference)
"""Cross-attention (B=4, Sq=4096, Sk=1024, H=16, D=1024) on 8 TRN2 NeuronCores.

Sharding: tensor-parallel by heads. Core c owns heads (2c, 2c+1), i.e. columns
[128c, 128c+128) of Wq/Wk/Wv and rows [128c, 128c+128) of Wo.

Per-core dataflow (all activations kept feature-on-partition, "transposed"):
  qT[c,i] = sum_k Wq[k,c] xT[k,i]        (lhsT=Wq chunk, rhs=xT chunk)
  kT[c,j] likewise from yT; v[j,d] natural layout (lhsT=yT chunk, rhs=Wv chunk)
  scoresT[j,i] = kT_h[:,j].T @ qT_h[:,i]  (per head, row-packed across 2 heads;
    the two K=64 matmuls land on PE row-groups (0,0)/(64,0) and stream together)
  e = exp(scoresT)  (no max-subtraction: scores are O(1) by construction)
  noutT[d,i] (+ sums row via an appended ones column in v_aug) accumulated over j
  attT = noutT * (1/sums) broadcast (DRAM-bounce broadcast DMA)
  AllToAll (per batch) head-shard -> seq-shard; out-proj on 512 rows/batch.

Precision: q-proj and k-proj run fp8e4 (inputs and weights x32, folded back in
the bias step) with perf_mode=DoubleRow, packing two 128-row K-chunks per
matmul for 2x PE throughput. Everything downstream of softmax (v, att, Wo, the
AllToAll payload) stays bf16: measured error contributions add in variance, and
the fp8 v/att/Wo paths would blow the 2e-2 rel-err budget (this config lands at
1.63e-2 vs 2.56e-3 all-bf16) while v-proj fp8 bought no speed at all.

Scheduling: three DMA rings are load-balanced (sync: xt/yt/sends/rv/output,
ACT: wv/ytv/wo, gpsimd: normalization bounces + collectives); the 2MB wo load
is deferred past the first batch's projection chains; out-proj of batch b-1 is
interleaved into the back half of batch b so only the final batch drains at the
end. Weight loads for the first matmuls are emitted before everything else.
"""

import numpy as np
import ml_dtypes

import concourse.bass as bass
import concourse.mybir as mybir
from concourse import bacc, tile
from concourse import bass_utils

BF16 = mybir.dt.bfloat16
F32 = mybir.dt.float32
FP8 = mybir.dt.float8e4
DR = mybir.MatmulPerfMode.DoubleRow

B = 4
SQ = 4096
SK = 1024
D = 1024
DC = 768
NCORES = 8
SQL = SQ // NCORES  # 512 output rows per batch per core
KC = D // 128       # 8 contraction chunks for q-proj / out-proj
FC = DC // 128      # 6 contraction chunks for k/v-proj
KC2 = KC // 2       # 4 DoubleRow (K=256) chunks for q-proj / out-proj
FC2 = FC // 2       # 3 DoubleRow chunks for k-proj
JC = SK // 128      # 8 key chunks
NI = SQ // 512      # 8 query blocks of 512 per batch

Exp = mybir.ActivationFunctionType.Exp
Alu = mybir.AluOpType


def build_nc():
    nc = bacc.Bacc(
        "TRN2",
        target_bir_lowering=False,
        debug=False,
        num_devices=NCORES,
    )

    xt = nc.dram_tensor("xt", [B, KC2, 128, 2, SQ], FP8, kind="ExternalInput")
    yt = nc.dram_tensor("yt", [B, FC2, 128, 2, SK], FP8, kind="ExternalInput")
    ytv = nc.dram_tensor("ytv", [B, FC, 128, SK], BF16, kind="ExternalInput")
    wq = nc.dram_tensor("wq", [KC2, 128, 2, 128], FP8, kind="ExternalInput")
    wk = nc.dram_tensor("wk", [FC2, 128, 2, 128], FP8, kind="ExternalInput")
    wv = nc.dram_tensor("wv", [FC, 128, 128], BF16, kind="ExternalInput")
    wo = nc.dram_tensor("wo", [KC, 128, D], BF16, kind="ExternalInput")
    bq = nc.dram_tensor("bq", [128, 1], F32, kind="ExternalInput")
    bk = nc.dram_tensor("bk", [128, 1], F32, kind="ExternalInput")
    bvb = nc.dram_tensor("bvb", [128, 128], F32, kind="ExternalInput")
    bob = nc.dram_tensor("bob", [128, D], F32, kind="ExternalInput")
    out = nc.dram_tensor("out", [B, SQL, D], F32, kind="ExternalOutput")

    # DRAM bounce buffers for the per-batch AllToAll.
    send = [
        nc.dram_tensor(f"a2a_send_{b}", [NCORES, 128, 512], BF16, kind="Internal")
        for b in range(B)
    ]
    recv = [
        nc.dram_tensor(f"a2a_recv_{b}", [NCORES, 128, 512], BF16, kind="Internal")
        for b in range(B)
    ]

    with tile.TileContext(nc) as tc:
        _program(nc, tc, xt, yt, ytv, wq, wk, wv, wo, bq, bk, bvb, bob, out, send, recv)
    nc.finalize()
    return nc


def _program(nc, tc, xt, yt, ytv, wq, wk, wv, wo, bq, bk, bvb, bob, out, send, recv):
    from contextlib import ExitStack

    with ExitStack() as ctx:
        const = ctx.enter_context(tc.tile_pool(name="const", bufs=1))
        ytp = ctx.enter_context(tc.tile_pool(name="ytp", bufs=7))
        ytvp = ctx.enter_context(tc.tile_pool(name="ytvp", bufs=13))
        xtp = ctx.enter_context(tc.tile_pool(name="xtp", bufs=13))
        bcp = ctx.enter_context(tc.tile_pool(name="bcp", bufs=5))
        nrp = ctx.enter_context(tc.tile_pool(name="nrp", bufs=6))
        qtp = ctx.enter_context(tc.tile_pool(name="qtp", bufs=2))
        ktp = ctx.enter_context(tc.tile_pool(name="ktp", bufs=2))
        vtp = ctx.enter_context(tc.tile_pool(name="vtp", bufs=16))
        ep = ctx.enter_context(tc.tile_pool(name="ep", bufs=3))
        recp = ctx.enter_context(tc.tile_pool(name="recp", bufs=4))
        attp = ctx.enter_context(tc.tile_pool(name="attp", bufs=6))
        rvp = ctx.enter_context(tc.tile_pool(name="rvp", bufs=9))
        outp = ctx.enter_context(tc.tile_pool(name="outp", bufs=2))
        # PSUM: scores 2x2 banks + nout 2x1 + proj 1 + outproj 1 = 8 banks
        scp = ctx.enter_context(tc.tile_pool(name="scp", bufs=2, space="PSUM"))
        noutp = ctx.enter_context(tc.tile_pool(name="noutp", bufs=2, space="PSUM"))
        projp = ctx.enter_context(tc.tile_pool(name="projp", bufs=1, space="PSUM"))
        outpp = ctx.enter_context(tc.tile_pool(name="outpp", bufs=1, space="PSUM"))
        rbp = ctx.enter_context(tc.tile_pool(name="rbp", bufs=6, space="DRAM"))

        # ---- constants / weights resident in SBUF. k-proj inputs load
        # first on the sync ring (they gate the first matmul); the v-proj
        # inputs (wv/bvb + ytv, loaded in emit_yt_loads) go on the ACT ring
        # so the two startup streams run in parallel.
        bk_sb = const.tile([128, 1], F32, tag="bk")
        nc.sync.dma_start(out=bk_sb[:, :], in_=bk[:, :])
        wk_sb = const.tile([128, FC2 * 256], FP8, tag="wk")
        for kk in range(FC2):
            nc.sync.dma_start(
                out=wk_sb[:, kk * 256:(kk + 1) * 256].rearrange(
                    "p (t m) -> p t m", t=2),
                in_=wk[kk, :, :, :],
            )
        bq_sb = const.tile([128, 1], F32, tag="bq")
        nc.sync.dma_start(out=bq_sb[:, :], in_=bq[:, :])
        wq_sb = const.tile([128, KC2 * 256], FP8, tag="wq")
        for kk in range(KC2):
            nc.sync.dma_start(
                out=wq_sb[:, kk * 256:(kk + 1) * 256].rearrange(
                    "p (t m) -> p t m", t=2),
                in_=wq[kk, :, :, :],
            )
        bvb_sb = const.tile([128, 128], F32, tag="bvb")
        nc.scalar.dma_start(out=bvb_sb[:, :], in_=bvb[:, :])
        bob_sb = const.tile([128, D], F32, tag="bob")
        wv_sb = const.tile([128, FC * 128], BF16, tag="wv")
        for fc in range(FC):
            nc.scalar.dma_start(
                out=wv_sb[:, fc * 128:(fc + 1) * 128], in_=wv[fc, :, :]
            )
        # wo (2MB) is first needed for batch-0 out-proj during batch 1's
        # attention; its load is emitted after the batch-0 projection chains
        # so it doesn't delay the first matmuls.
        wo_sb = const.tile([128, KC * D], BF16, tag="wo")

        def emit_wo_load():
            nc.scalar.dma_start(out=bob_sb[:, :], in_=bob[:, :])
            for kc in range(KC):
                nc.scalar.dma_start(
                    out=wo_sb[:, kc * D:(kc + 1) * D], in_=wo[kc, :, :]
                )

        v_tiles = {}
        rv_tiles = {}

        def emit_rv_loads(rb_):
            rvs = []
            for cc in range(KC):
                rv = rvp.tile([128, 512], BF16, name=f"rv_{rb_}_{cc}", tag="rv")
                nc.sync.dma_start(out=rv[:, :], in_=recv[rb_][cc, :, :])
                rvs.append(rv)
            rv_tiles[rb_] = rvs

        def emit_outproj_chunk(ob, chunk, pool=None, tag="ops"):
            # output projection for batch ob, one (i1, eh) chunk of 8
            i1, eh = divmod(chunk, 2)
            rvs = rv_tiles[ob]
            ops = (pool or outpp).tile(
                [128, 512], F32, name=f"ops_{ob}_{i1}_{eh}", tag=tag
            )
            for cc in range(KC):
                nc.tensor.matmul(
                    ops[:, :],
                    lhsT=rvs[cc][:, i1 * 128:(i1 + 1) * 128],
                    rhs=wo_sb[:, cc * D + eh * 512: cc * D + (eh + 1) * 512],
                    start=(cc == 0),
                    stop=(cc == KC - 1),
                )
            o_t = outp.tile([128, 512], F32, name=f"o_{ob}_{i1}_{eh}", tag="o")
            nc.vector.tensor_add(
                o_t[:, :], ops[:, :], bob_sb[:, eh * 512:(eh + 1) * 512]
            )
            nc.sync.dma_start(
                out=out[ob, i1 * 128:(i1 + 1) * 128, eh * 512:(eh + 1) * 512],
                in_=o_t[:, :],
            )

        yts_d = {}
        ytvs_d = {}
        kt_d = {}
        qt_d = {}

        def emit_yt_loads(pb):
            yts = []
            for fc2 in range(FC2):
                yt_t = ytp.tile([128, 2 * SK], FP8, name=f"yt_{pb}_{fc2}", tag="yt")
                nc.sync.dma_start(
                    out=yt_t[:, :].rearrange("p (t s) -> p t s", t=2),
                    in_=yt[pb, fc2, :, :, :],
                )
                yts.append(yt_t)
            ytvs = []
            for fc in range(FC):
                ytv_t = ytvp.tile([128, SK], BF16, name=f"ytv_{pb}_{fc}",
                                  tag="ytv")
                nc.scalar.dma_start(out=ytv_t[:, :], in_=ytv[pb, fc, :, :])
                ytvs.append(ytv_t)
            ytvs_d[pb] = ytvs
            yts_d[pb] = yts
            kt_d[pb] = ktp.tile([128, SK], BF16, name=f"kt_{pb}", tag="kt")
            qt_d[pb] = qtp.tile([128, SQ], BF16, name=f"qt_{pb}", tag="qt")

        def emit_k_chain(pb, j2):
            yts = yts_d[pb]
            kps = projp.tile([128, 512], F32, name=f"kps_{pb}_{j2}", tag="proj")
            for fc2 in range(FC2):
                nc.tensor.matmul(
                    kps[:, :],
                    lhsT=wk_sb[:, fc2 * 256:(fc2 + 1) * 256].rearrange(
                        "p (t m) -> p t m", t=2),
                    rhs=yts[fc2][:, :].rearrange(
                        "p (t s) -> p t s", t=2)[:, :, j2 * 512:(j2 + 1) * 512],
                    perf_mode=DR,
                    start=(fc2 == 0),
                    stop=(fc2 == FC2 - 1),
                )
            # psum holds 32*(y @ Wk); bk_sb holds 32*bk
            nc.vector.tensor_scalar(
                out=kt_d[pb][:, j2 * 512:(j2 + 1) * 512], in0=kps[:, :],
                scalar1=bk_sb[:, :], scalar2=1.0 / 32.0,
                op0=Alu.add, op1=Alu.mult,
            )

        def emit_v_chain(pb, jc):
            # v_aug layout per tile [128, 130]:
            #   cols 0:64   = head-A values, col 64 = ones (A sums)
            #   cols 65:129 = head-B values, col 129 = ones (B sums)
            ytvs = ytvs_d[pb]
            vps = projp.tile([128, 128], F32, name=f"vps_{pb}_{jc}", tag="proj")
            for fc in range(FC):
                nc.tensor.matmul(
                    vps[:, :],
                    lhsT=ytvs[fc][:, jc * 128:(jc + 1) * 128],
                    rhs=wv_sb[:, fc * 128:(fc + 1) * 128],
                    start=(fc == 0),
                    stop=(fc == FC - 1),
                )
            v_t = vtp.tile([128, 130], BF16, name=f"v_{pb}_{jc}", tag="vt")
            nc.vector.tensor_tensor(
                out=v_t[:, 0:130].rearrange("p (h x) -> p h x", h=2)[:, :, 0:64],
                in0=vps[:, :].rearrange("p (h x) -> p h x", h=2),
                in1=bvb_sb[:, :].rearrange("p (h x) -> p h x", h=2),
                op=Alu.add,
            )
            nc.vector.memset(v_t[:, 64:65], 1.0)
            nc.vector.memset(v_t[:, 129:130], 1.0)
            v_tiles[(pb, jc)] = v_t

        xt_tiles = {}

        def emit_xt_loads(pb, g):
            for kk in range(KC2):
                t = xtp.tile([128, 2048], FP8, name=f"xt_{pb}_{g}_{kk}", tag="xt")
                nc.sync.dma_start(
                    out=t[:, :].rearrange("p (t s) -> p t s", t=2),
                    in_=xt[pb, kk, :, :, g * 1024:(g + 1) * 1024],
                )
                xt_tiles[(g, kk)] = t

        def emit_q_chain(pb, i5):
            qps = projp.tile([128, 512], F32, name=f"qps_{pb}_{i5}", tag="proj")
            h5 = i5 % 2
            for kk in range(KC2):
                nc.tensor.matmul(
                    qps[:, :],
                    lhsT=wq_sb[:, kk * 256:(kk + 1) * 256].rearrange(
                        "p (t m) -> p t m", t=2),
                    rhs=xt_tiles[(i5 // 2, kk)][:, :].rearrange(
                        "p (t s) -> p t s", t=2)[:, :, h5 * 512:(h5 + 1) * 512],
                    perf_mode=DR,
                    start=(kk == 0),
                    stop=(kk == KC2 - 1),
                )
            # psum holds 32*(x @ Wq); bq_sb holds 32*bq
            nc.vector.tensor_scalar(
                out=qt_d[pb][:, i5 * 512:(i5 + 1) * 512],
                in0=qps[:, :],
                scalar1=bq_sb[:, :],
                scalar2=0.125 / 32.0,
                op0=Alu.add,
                op1=Alu.mult,
            )

        # ---- startup: batch 0 projections
        emit_yt_loads(0)
        for j2 in range(SK // 512):
            emit_k_chain(0, j2)
        for jc in range(JC):
            emit_v_chain(0, jc)
        for g in range(NI // 2):
            emit_xt_loads(0, g)
        for i5 in range(NI):
            emit_q_chain(0, i5)

        norm_q = []
        LAG = 2

        def emit_mul_send(mb, mi5, nsb, bc):
            att = attp.tile([64, 1024], BF16, name=f"att_{mb}_{mi5}", tag="att")
            nc.vector.tensor_mul(att[:, :], nsb[0:64, :], bc[:, :])
            for h in range(2):
                nc.sync.dma_start(
                    out=send[mb][mi5, h * 64:(h + 1) * 64, :],
                    in_=att[:, h * 512:(h + 1) * 512],
                )

        for b in range(B):
            kt_sb = kt_d[b]
            qt_sb = qt_d[b]
            if b + 1 < B:
                emit_yt_loads(b + 1)
            if b == 0:
                emit_wo_load()

            # ---- attention, one 512-wide query block at a time, with the
            # next batch's projections and the previous batch's output
            # projection interleaved to keep the PE stream dense
            for i5 in range(NI):
                isl = slice(i5 * 512, (i5 + 1) * 512)
                na = noutp.tile([65, 512], F32, name=f"na_{b}_{i5}", tag="nout")
                nb = noutp.tile([65, 512], F32, name=f"nb_{b}_{i5}", tag="nout")
                for jc in range(JC):
                    sc = scp.tile([128, 1024], F32, name=f"sc_{b}_{i5}_{jc}", tag="sc")
                    jsl = slice(jc * 128, (jc + 1) * 128)
                    # scoresT for both heads, row-packed (K=64 each)
                    nc.tensor.matmul(
                        sc[:, 0:512],
                        lhsT=kt_sb[0:64, jsl],
                        rhs=qt_sb[0:64, isl],
                        start=True, stop=True,
                    )
                    nc.tensor.matmul(
                        sc[:, 512:1024],
                        lhsT=kt_sb[64:128, jsl],
                        rhs=qt_sb[64:128, isl],
                        start=True, stop=True,
                    )
                    e_t = ep.tile([128, 1024], BF16, name=f"e_{b}_{i5}_{jc}", tag="e")
                    nc.scalar.activation(e_t[:, :], sc[:, :], Exp)
                    v_t = v_tiles[(b, jc)]
                    nc.tensor.matmul(
                        na[:, :],
                        lhsT=v_t[:, 0:65],
                        rhs=e_t[:, 0:512],
                        start=(jc == 0),
                        stop=(jc == JC - 1),
                    )
                    nc.tensor.matmul(
                        nb[:, :],
                        lhsT=v_t[:, 65:130],
                        rhs=e_t[:, 512:1024],
                        start=(jc == 0),
                        stop=(jc == JC - 1),
                    )
                # normalize + emit to the A2A send buffer. Stage 1 per
                # block: psum evac, reciprocal, DRAM bounce broadcast.
                # Stage 2 (mul + send) runs LAG blocks later so a starved
                # bc DMA (A2A transfer window) can't stall the vector FIFO.
                nsb = nrp.tile([65, 1024], F32, name=f"nsb_{b}_{i5}", tag="nr")
                nc.vector.tensor_copy(nsb[:, 0:512], na[:, :])
                nc.vector.tensor_copy(nsb[:, 512:1024], nb[:, :])
                rec = recp.tile([65, 1024], F32, name=f"rec_{b}_{i5}", tag="rec")
                # NB: base_partition must be 0 for the custom DVE op
                # (row-64-only slices produce garbage on HW), so compute
                # 1/x over the whole tile and use just the sums row.
                nc.vector.reciprocal_approx_fast(out=rec[:, :], in_=nsb[:, :])
                # DRAM-bounce broadcast on the sync ring: keeps the bounce
                # off the gpsimd ring, where it would queue behind the
                # previous batch's AllToAll and stall every mul behind it
                rb = rbp.tile([1, 1024], F32, name=f"rb_{b}_{i5}", tag="rb")
                nc.sync.dma_start(out=rb[:, :], in_=rec[64:65, :])
                bc = bcp.tile([64, 1024], F32, name=f"bc_{b}_{i5}", tag="bc")
                nc.sync.dma_start(
                    out=bc[:, :], in_=rb[0:1, :].to_broadcast([64, 1024])
                )
                norm_q.append((b, i5, nsb, bc))
                if len(norm_q) > LAG:
                    emit_mul_send(*norm_q.pop(0))
                if b + 1 < B:
                    if i5 == 0:
                        emit_xt_loads(b + 1, 0)
                        emit_xt_loads(b + 1, 1)
                    elif i5 in (2, 4):
                        emit_xt_loads(b + 1, i5 // 2 + 1)
                    # chain MMs only in the back half: during blocks 0-2 the
                    # previous batch's AllToAll transfer starves the DMA
                    # queues, and a stalled chain MM blocks the in-order PE
                    # queue (attention itself is SBUF-resident there).
                    for vj in {3: (0, 1), 4: (2, 3), 5: (4, 5), 6: (6, 7)}.get(i5, ()):
                        emit_v_chain(b + 1, vj)
                    if i5 in (3, 4):
                        emit_k_chain(b + 1, i5 - 3)
                    for qj in {5: (0, 1), 6: (2, 3), 7: (4, 5, 6, 7)}.get(i5, ()):
                        emit_q_chain(b + 1, qj)
                # previous batch: its AllToAll (issued at the end of batch
                # b-1) completes ~22us in; load its rv tiles at i5==2 and
                # run its out-proj during the back half of this batch
                if b > 0 and i5 == 4:
                    emit_rv_loads(b - 1)
                if b > 0 and i5 >= 5:
                    n_chunks = [3, 3, 2][i5 - 5]
                    base = [0, 3, 6][i5 - 5]
                    for ch in range(n_chunks):
                        emit_outproj_chunk(b - 1, base + ch)

            while norm_q:
                emit_mul_send(*norm_q.pop(0))
            # ---- AllToAll for this batch: head-shard -> seq-shard
            nc.gpsimd.collective_compute(
                "AllToAll",
                Alu.bypass,
                replica_groups=[list(range(NCORES))],
                ins=[send[b][:, :, :].opt()],
                outs=[recv[b][:, :, :].opt()],
            )

        # keep the PE's HAM clock warm through the final AllToAll wait:
        # fp32 matmuls on resident data run at 1/4 rate (~850ns each), so a
        # dozen of them span ~10us of the ~16us window; without them the
        # out-proj drain starts at the cold 1.2 GHz clock.
        for w in range(20):
            warm = projp.tile([128, 512], F32, name=f"warm_{w}", tag="proj")
            nc.tensor.matmul(
                warm[:, :],
                lhsT=bob_sb[:, 0:128],
                rhs=bob_sb[:, 0:512],
                start=True, stop=True,
            )
        # drain the last batch's output projections, rotating over the
        # now-idle psum pools for pipelining depth
        emit_rv_loads(B - 1)
        for chunk in range(8):
            pool, tag = [(outpp, "ops"), (scp, "sc"),
                         (noutp, "nout"), (scp, "sc")][chunk % 4]
            emit_outproj_chunk(B - 1, chunk, pool=pool, tag=tag)


def _f8(a):
    return np.clip(a, -240.0, 240.0).astype(ml_dtypes.float8_e4m3)


def prep_in_maps(x, y, Wq, bq, Wk, bk, Wv, bv, Wo, bo):
    x = np.asarray(x, np.float32)
    y = np.asarray(y, np.float32)
    # paired fp8 layouts for DoubleRow: [.., kk, p, t, ..] with feature
    # index f = (2*kk + t)*128 + p
    xT = x.transpose(0, 2, 1)                       # [B, D, SQ]
    xt = _f8(np.ascontiguousarray(
        xT.reshape(B, KC2, 2, 128, SQ).transpose(0, 1, 3, 2, 4)))
    yT = y.transpose(0, 2, 1)                       # [B, DC, SK]
    yt = _f8(np.ascontiguousarray(
        yT.reshape(B, FC2, 2, 128, SK).transpose(0, 1, 3, 2, 4)))
    ytv = np.ascontiguousarray(
        yT.reshape(B, FC, 128, SK)).astype(ml_dtypes.bfloat16)
    bf = ml_dtypes.bfloat16
    wo = np.ascontiguousarray(
        np.asarray(Wo, np.float32).reshape(KC, 128, D)).astype(bf)
    bob = np.ascontiguousarray(
        np.broadcast_to(np.asarray(bo, np.float32)[None, :], (128, D))
    )
    in_maps = []
    for c in range(NCORES):
        cs = slice(c * 128, (c + 1) * 128)
        wq_c = np.asarray(Wq, np.float32)[:, cs] * 32.0
        wk_c = np.asarray(Wk, np.float32)[:, cs] * 32.0
        wv_c = np.asarray(Wv, np.float32)[:, cs]
        in_maps.append({
            "xt": xt,
            "yt": yt,
            "ytv": ytv,
            "wq": _f8(np.ascontiguousarray(
                wq_c.reshape(KC2, 2, 128, 128).transpose(0, 2, 1, 3))),
            "wk": _f8(np.ascontiguousarray(
                wk_c.reshape(FC2, 2, 128, 128).transpose(0, 2, 1, 3))),
            "wv": np.ascontiguousarray(wv_c.reshape(FC, 128, 128)).astype(bf),
            "wo": wo,
            "bq": np.ascontiguousarray(
                np.asarray(bq, np.float32)[cs].reshape(128, 1) * 32.0),
            "bk": np.ascontiguousarray(
                np.asarray(bk, np.float32)[cs].reshape(128, 1) * 32.0),
            "bvb": np.ascontiguousarray(
                np.broadcast_to(np.asarray(bv, np.float32)[cs][None, :], (128, 128))
            ),
            "bob": bob,
        })
    return in_maps


_NC_CACHE = None


def get_nc():
    global _NC_CACHE
    if _NC_CACHE is None:
        _NC_CACHE = build_nc()
    return _NC_CACHE


def run(in_maps, **kwargs):
    nc = get_nc()
    return bass_utils.run_bass_kernel_spmd(
        nc, in_maps, core_ids=list(range(NCORES)), **kwargs
    )


def gather(results):
    full = np.empty((B, SQ, D), np.float32)
    for c in range(NCORES):
        full[:, c * SQL:(c + 1) * SQL, :] = results[c]["out"]
    return full


def kernel(**inputs):
    in_maps = prep_in_maps(**inputs)
    res = run(in_maps)
    return gather(res.results)


if __name__ == "__main__":
    nc = build_nc()
    print("build OK")



# revision 34
# speedup vs baseline: 1.0554x; 1.0554x over previous
"""Cross-attention (B=4, Sq=4096, Sk=1024, H=16, D=1024) on 8 TRN2 NeuronCores.

Sharding: tensor-parallel by heads. Core c owns heads (2c, 2c+1), i.e. columns
[128c, 128c+128) of Wq/Wk/Wv and rows [128c, 128c+128) of Wo.

Per-core dataflow (all activations kept feature-on-partition, "transposed"):
  qT[c,i] = sum_k Wq[k,c] xT[k,i]        (lhsT=Wq chunk, rhs=xT chunk)
  kT[c,j] likewise from yT; v[j,d] natural layout (lhsT=yT chunk, rhs=Wv chunk)
  scoresT[j,i] = kT_h[:,j].T @ qT_h[:,i]  (per head, row-packed across 2 heads;
    the two K=64 matmuls land on PE row-groups (0,0)/(64,0) and stream together)
  e = exp(scoresT)  (no max-subtraction: scores are O(1) by construction)
  noutT[d,i] (+ sums row via an appended ones column in v_aug) accumulated over j
  attT = noutT * (1/sums) broadcast (DRAM-bounce broadcast DMA)
  AllToAll (per batch) head-shard -> seq-shard; out-proj on 512 rows/batch.

Precision: q-proj and k-proj run fp8e4 (inputs and weights x32, folded back in
the bias step) with perf_mode=DoubleRow, packing two 128-row K-chunks per
matmul for 2x PE throughput. Everything downstream of softmax (v, att, Wo, the
AllToAll payload) stays bf16: measured error contributions add in variance, and
the fp8 v/att/Wo paths would blow the 2e-2 rel-err budget (this config lands at
1.63e-2 vs 2.56e-3 all-bf16) while v-proj fp8 bought no speed at all.

Scheduling: three DMA rings are load-balanced (sync: xt/yt/sends/rv/output,
ACT: wv/ytv/wo, gpsimd: normalization bounces + collectives); the 2MB wo load
is deferred past the first batch's projection chains; out-proj of batch b-1 is
interleaved into the back half of batch b so only the final batch drains at the
end. Weight loads for the first matmuls are emitted before everything else.
"""

import numpy as np
import ml_dtypes

import concourse.bass as bass
import concourse.mybir as mybir
from concourse import bacc, tile
from concourse import bass_utils

BF16 = mybir.dt.bfloat16
F32 = mybir.dt.float32
FP8 = mybir.dt.float8e4
DR = mybir.MatmulPerfMode.DoubleRow

B = 4
SQ = 4096
SK = 1024
D = 1024
DC = 768
NCORES = 8
SQL = SQ // NCORES  # 512 output rows per batch per core
KC = D // 128       # 8 contraction chunks for q-proj / out-proj
FC = DC // 128      # 6 contraction chunks for k/v-proj
KC2 = KC // 2       # 4 DoubleRow (K=256) chunks for q-proj / out-proj
FC2 = FC // 2       # 3 DoubleRow chunks for k-proj
JC = SK // 128      # 8 key chunks
NI = SQ // 512      # 8 query blocks of 512 per batch

Exp = mybir.ActivationFunctionType.Exp
Alu = mybir.AluOpType


def build_nc():
    nc = bacc.Bacc(
        "TRN2",
        target_bir_lowering=False,
        debug=False,
        num_devices=NCORES,
    )

    xt = nc.dram_tensor("xt", [B, KC2, 128, 2, SQ], FP8, kind="ExternalInput")
    yt = nc.dram_tensor("yt", [B, FC2, 128, 2, SK], FP8, kind="ExternalInput")
    ytv = nc.dram_tensor("ytv", [B, FC, 128, SK], BF16, kind="ExternalInput")
    wq = nc.dram_tensor("wq", [KC2, 128, 2, 128], FP8, kind="ExternalInput")
    wk = nc.dram_tensor("wk", [FC2, 128, 2, 128], FP8, kind="ExternalInput")
    wv = nc.dram_tensor("wv", [FC, 128, 128], BF16, kind="ExternalInput")
    wo = nc.dram_tensor("wo", [KC, 128, D], BF16, kind="ExternalInput")
    bq = nc.dram_tensor("bq", [128, 1], F32, kind="ExternalInput")
    bk = nc.dram_tensor("bk", [128, 1], F32, kind="ExternalInput")
    bvb = nc.dram_tensor("bvb", [128, 128], F32, kind="ExternalInput")
    bob = nc.dram_tensor("bob", [128, D], F32, kind="ExternalInput")
    out = nc.dram_tensor("out", [B, SQL, D], F32, kind="ExternalOutput")

    # DRAM bounce buffers for the per-batch AllToAll.
    send = [
        nc.dram_tensor(f"a2a_send_{b}", [NCORES, 128, 512], BF16, kind="Internal")
        for b in range(B)
    ]
    recv = [
        nc.dram_tensor(f"a2a_recv_{b}", [NCORES, 128, 512], BF16, kind="Internal")
        for b in range(B)
    ]

    with tile.TileContext(nc) as tc:
        _program(nc, tc, xt, yt, ytv, wq, wk, wv, wo, bq, bk, bvb, bob, out, send, recv)
    nc.finalize()
    return nc


def _program(nc, tc, xt, yt, ytv, wq, wk, wv, wo, bq, bk, bvb, bob, out, send, recv):
    from contextlib import ExitStack

    with ExitStack() as ctx:
        const = ctx.enter_context(tc.tile_pool(name="const", bufs=1))
        ytp = ctx.enter_context(tc.tile_pool(name="ytp", bufs=7))
        ytvp = ctx.enter_context(tc.tile_pool(name="ytvp", bufs=13))
        xtp = ctx.enter_context(tc.tile_pool(name="xtp", bufs=13))
        bcp = ctx.enter_context(tc.tile_pool(name="bcp", bufs=5))
        nrp = ctx.enter_context(tc.tile_pool(name="nrp", bufs=6))
        qtp = ctx.enter_context(tc.tile_pool(name="qtp", bufs=2))
        ktp = ctx.enter_context(tc.tile_pool(name="ktp", bufs=2))
        vtp = ctx.enter_context(tc.tile_pool(name="vtp", bufs=16))
        ep = ctx.enter_context(tc.tile_pool(name="ep", bufs=3))
        recp = ctx.enter_context(tc.tile_pool(name="recp", bufs=4))
        attp = ctx.enter_context(tc.tile_pool(name="attp", bufs=6))
        rvp = ctx.enter_context(tc.tile_pool(name="rvp", bufs=9))
        outp = ctx.enter_context(tc.tile_pool(name="outp", bufs=2))
        # PSUM: scores 2x2 banks + nout 2x1 + proj 1 + outproj 1 = 8 banks
        scp = ctx.enter_context(tc.tile_pool(name="scp", bufs=2, space="PSUM"))
        noutp = ctx.enter_context(tc.tile_pool(name="noutp", bufs=2, space="PSUM"))
        projp = ctx.enter_context(tc.tile_pool(name="projp", bufs=1, space="PSUM"))
        outpp = ctx.enter_context(tc.tile_pool(name="outpp", bufs=1, space="PSUM"))
        rbp = ctx.enter_context(tc.tile_pool(name="rbp", bufs=6, space="DRAM"))

        # ---- constants / weights resident in SBUF. k-proj inputs load
        # first on the sync ring (they gate the first matmul); the v-proj
        # inputs (wv/bvb + ytv, loaded in emit_yt_loads) go on the ACT ring
        # so the two startup streams run in parallel.
        bk_sb = const.tile([128, 1], F32, tag="bk")
        nc.sync.dma_start(out=bk_sb[:, :], in_=bk[:, :])
        wk_sb = const.tile([128, FC2 * 256], FP8, tag="wk")
        for kk in range(FC2):
            nc.sync.dma_start(
                out=wk_sb[:, kk * 256:(kk + 1) * 256].rearrange(
                    "p (t m) -> p t m", t=2),
                in_=wk[kk, :, :, :],
            )
        bq_sb = const.tile([128, 1], F32, tag="bq")
        nc.sync.dma_start(out=bq_sb[:, :], in_=bq[:, :])
        wq_sb = const.tile([128, KC2 * 256], FP8, tag="wq")
        for kk in range(KC2):
            nc.sync.dma_start(
                out=wq_sb[:, kk * 256:(kk + 1) * 256].rearrange(
                    "p (t m) -> p t m", t=2),
                in_=wq[kk, :, :, :],
            )
        bvb_sb = const.tile([128, 128], F32, tag="bvb")
        nc.scalar.dma_start(out=bvb_sb[:, :], in_=bvb[:, :])
        bob_sb = const.tile([128, D], F32, tag="bob")
        wv_sb = const.tile([128, FC * 128], BF16, tag="wv")
        for fc in range(FC):
            nc.scalar.dma_start(
                out=wv_sb[:, fc * 128:(fc + 1) * 128], in_=wv[fc, :, :]
            )
        # wo (2MB) is first needed for batch-0 out-proj during batch 1's
        # attention; its load is emitted after the batch-0 projection chains
        # so it doesn't delay the first matmuls.
        wo_sb = const.tile([128, KC * D], BF16, tag="wo")

        def emit_wo_load():
            nc.scalar.dma_start(out=bob_sb[:, :], in_=bob[:, :])
            for kc in range(KC):
                nc.scalar.dma_start(
                    out=wo_sb[:, kc * D:(kc + 1) * D], in_=wo[kc, :, :]
                )

        v_tiles = {}
        rv_tiles = {}

        def emit_rv_loads(rb_):
            rvs = []
            for cc in range(KC):
                rv = rvp.tile([128, 512], BF16, name=f"rv_{rb_}_{cc}", tag="rv")
                nc.sync.dma_start(out=rv[:, :], in_=recv[rb_][cc, :, :])
                rvs.append(rv)
            rv_tiles[rb_] = rvs

        def emit_outproj_chunk(ob, chunk, pool=None, tag="ops"):
            # output projection for batch ob, one (i1, eh) chunk of 8
            i1, eh = divmod(chunk, 2)
            rvs = rv_tiles[ob]
            ops = (pool or outpp).tile(
                [128, 512], F32, name=f"ops_{ob}_{i1}_{eh}", tag=tag
            )
            for cc in range(KC):
                nc.tensor.matmul(
                    ops[:, :],
                    lhsT=rvs[cc][:, i1 * 128:(i1 + 1) * 128],
                    rhs=wo_sb[:, cc * D + eh * 512: cc * D + (eh + 1) * 512],
                    start=(cc == 0),
                    stop=(cc == KC - 1),
                )
            o_t = outp.tile([128, 512], F32, name=f"o_{ob}_{i1}_{eh}", tag="o")
            nc.vector.tensor_add(
                o_t[:, :], ops[:, :], bob_sb[:, eh * 512:(eh + 1) * 512]
            )
            nc.sync.dma_start(
                out=out[ob, i1 * 128:(i1 + 1) * 128, eh * 512:(eh + 1) * 512],
                in_=o_t[:, :],
            )

        yts_d = {}
        ytvs_d = {}
        kt_d = {}
        qt_d = {}

        def emit_yt_loads(pb):
            yts = []
            for fc2 in range(FC2):
                yt_t = ytp.tile([128, 2 * SK], FP8, name=f"yt_{pb}_{fc2}", tag="yt")
                nc.sync.dma_start(
                    out=yt_t[:, :].rearrange("p (t s) -> p t s", t=2),
                    in_=yt[pb, fc2, :, :, :],
                )
                yts.append(yt_t)
            ytvs = []
            for fc in range(FC):
                ytv_t = ytvp.tile([128, SK], BF16, name=f"ytv_{pb}_{fc}",
                                  tag="ytv")
                nc.scalar.dma_start(out=ytv_t[:, :], in_=ytv[pb, fc, :, :])
                ytvs.append(ytv_t)
            ytvs_d[pb] = ytvs
            yts_d[pb] = yts
            kt_d[pb] = ktp.tile([128, SK], BF16, name=f"kt_{pb}", tag="kt")
            qt_d[pb] = qtp.tile([128, SQ], BF16, name=f"qt_{pb}", tag="qt")

        def emit_k_chain(pb, j2):
            yts = yts_d[pb]
            kps = projp.tile([128, 512], F32, name=f"kps_{pb}_{j2}", tag="proj")
            for fc2 in range(FC2):
                nc.tensor.matmul(
                    kps[:, :],
                    lhsT=wk_sb[:, fc2 * 256:(fc2 + 1) * 256].rearrange(
                        "p (t m) -> p t m", t=2),
                    rhs=yts[fc2][:, :].rearrange(
                        "p (t s) -> p t s", t=2)[:, :, j2 * 512:(j2 + 1) * 512],
                    perf_mode=DR,
                    start=(fc2 == 0),
                    stop=(fc2 == FC2 - 1),
                )
            # psum holds 32*(y @ Wk); bk_sb holds 32*bk
            nc.vector.tensor_scalar(
                out=kt_d[pb][:, j2 * 512:(j2 + 1) * 512], in0=kps[:, :],
                scalar1=bk_sb[:, :], scalar2=1.0 / 32.0,
                op0=Alu.add, op1=Alu.mult,
            )

        def emit_v_chain(pb, jc):
            # v_aug layout per tile [128, 130]:
            #   cols 0:64   = head-A values, col 64 = ones (A sums)
            #   cols 65:129 = head-B values, col 129 = ones (B sums)
            ytvs = ytvs_d[pb]
            vps = projp.tile([128, 128], F32, name=f"vps_{pb}_{jc}", tag="proj")
            for fc in range(FC):
                nc.tensor.matmul(
                    vps[:, :],
                    lhsT=ytvs[fc][:, jc * 128:(jc + 1) * 128],
                    rhs=wv_sb[:, fc * 128:(fc + 1) * 128],
                    start=(fc == 0),
                    stop=(fc == FC - 1),
                )
            v_t = vtp.tile([128, 130], BF16, name=f"v_{pb}_{jc}", tag="vt")
            nc.vector.tensor_tensor(
                out=v_t[:, 0:130].rearrange("p (h x) -> p h x", h=2)[:, :, 0:64],
                in0=vps[:, :].rearrange("p (h x) -> p h x", h=2),
                in1=bvb_sb[:, :].rearrange("p (h x) -> p h x", h=2),
                op=Alu.add,
            )
            nc.vector.memset(v_t[:, 64:65], 1.0)
            nc.vector.memset(v_t[:, 129:130], 1.0)
            v_tiles[(pb, jc)] = v_t

        xt_tiles = {}

        def emit_xt_loads(pb, g):
            for kk in range(KC2):
                t = xtp.tile([128, 2048], FP8, name=f"xt_{pb}_{g}_{kk}", tag="xt")
                nc.sync.dma_start(
                    out=t[:, :].rearrange("p (t s) -> p t s", t=2),
                    in_=xt[pb, kk, :, :, g * 1024:(g + 1) * 1024],
                )
                xt_tiles[(g, kk)] = t

        def emit_q_chain(pb, i5):
            qps = projp.tile([128, 512], F32, name=f"qps_{pb}_{i5}", tag="proj")
            h5 = i5 % 2
            for kk in range(KC2):
                nc.tensor.matmul(
                    qps[:, :],
                    lhsT=wq_sb[:, kk * 256:(kk + 1) * 256].rearrange(
                        "p (t m) -> p t m", t=2),
                    rhs=xt_tiles[(i5 // 2, kk)][:, :].rearrange(
                        "p (t s) -> p t s", t=2)[:, :, h5 * 512:(h5 + 1) * 512],
                    perf_mode=DR,
                    start=(kk == 0),
                    stop=(kk == KC2 - 1),
                )
            # psum holds 32*(x @ Wq); bq_sb holds 32*bq
            nc.vector.tensor_scalar(
                out=qt_d[pb][:, i5 * 512:(i5 + 1) * 512],
                in0=qps[:, :],
                scalar1=bq_sb[:, :],
                scalar2=0.125 / 32.0,
                op0=Alu.add,
                op1=Alu.mult,
            )

        # ---- startup: batch 0 projections
        emit_yt_loads(0)
        for j2 in range(SK // 512):
            emit_k_chain(0, j2)
        for jc in range(JC):
            emit_v_chain(0, jc)
        for g in range(NI // 2):
            emit_xt_loads(0, g)
        for i5 in range(NI):
            emit_q_chain(0, i5)

        norm_q = []
        LAG = 2

        def emit_mul_send(mb, mi5, nsb, bc):
            att = attp.tile([64, 1024], BF16, name=f"att_{mb}_{mi5}", tag="att")
            nc.vector.tensor_mul(att[:, :], nsb[0:64, :], bc[:, :])
            for h in range(2):
                nc.sync.dma_start(
                    out=send[mb][mi5, h * 64:(h + 1) * 64, :],
                    in_=att[:, h * 512:(h + 1) * 512],
                )

        for b in range(B):
            kt_sb = kt_d[b]
            qt_sb = qt_d[b]
            if b + 1 < B:
                emit_yt_loads(b + 1)
            if b == 0:
                emit_wo_load()

            # ---- attention, one 512-wide query block at a time, with the
            # next batch's projections and the previous batch's output
            # projection interleaved to keep the PE stream dense
            for i5 in range(NI):
                isl = slice(i5 * 512, (i5 + 1) * 512)
                na = noutp.tile([65, 512], F32, name=f"na_{b}_{i5}", tag="nout")
                nb = noutp.tile([65, 512], F32, name=f"nb_{b}_{i5}", tag="nout")
                for jc in range(JC):
                    sc = scp.tile([128, 1024], F32, name=f"sc_{b}_{i5}_{jc}", tag="sc")
                    jsl = slice(jc * 128, (jc + 1) * 128)
                    # scoresT for both heads, row-packed (K=64 each)
                    nc.tensor.matmul(
                        sc[:, 0:512],
                        lhsT=kt_sb[0:64, jsl],
                        rhs=qt_sb[0:64, isl],
                        start=True, stop=True,
                    )
                    nc.tensor.matmul(
                        sc[:, 512:1024],
                        lhsT=kt_sb[64:128, jsl],
                        rhs=qt_sb[64:128, isl],
                        start=True, stop=True,
                    )
                    e_t = ep.tile([128, 1024], BF16, name=f"e_{b}_{i5}_{jc}", tag="e")
                    nc.scalar.activation(e_t[:, :], sc[:, :], Exp)
                    v_t = v_tiles[(b, jc)]
                    nc.tensor.matmul(
                        na[:, :],
                        lhsT=v_t[:, 0:65],
                        rhs=e_t[:, 0:512],
                        start=(jc == 0),
                        stop=(jc == JC - 1),
                    )
                    nc.tensor.matmul(
                        nb[:, :],
                        lhsT=v_t[:, 65:130],
                        rhs=e_t[:, 512:1024],
                        start=(jc == 0),
                        stop=(jc == JC - 1),
                    )
                # normalize + emit to the A2A send buffer. Stage 1 per
                # block: psum evac, reciprocal, DRAM bounce broadcast.
                # Stage 2 (mul + send) runs LAG blocks later so a starved
                # bc DMA (A2A transfer window) can't stall the vector FIFO.
                nsb = nrp.tile([65, 1024], F32, name=f"nsb_{b}_{i5}", tag="nr")
                nc.vector.tensor_copy(nsb[:, 0:512], na[:, :])
                nc.vector.tensor_copy(nsb[:, 512:1024], nb[:, :])
                rec = recp.tile([65, 1024], F32, name=f"rec_{b}_{i5}", tag="rec")
                # NB: base_partition must be 0 for the custom DVE op
                # (row-64-only slices produce garbage on HW), so compute
                # 1/x over the whole tile and use just the sums row.
                nc.vector.reciprocal_approx_fast(out=rec[:, :], in_=nsb[:, :])
                rb = rbp.tile([1, 1024], F32, name=f"rb_{b}_{i5}", tag="rb")
                nc.gpsimd.dma_start(out=rb[:, :], in_=rec[64:65, :])
                bc = bcp.tile([64, 1024], F32, name=f"bc_{b}_{i5}", tag="bc")
                nc.gpsimd.dma_start(
                    out=bc[:, :], in_=rb[0:1, :].to_broadcast([64, 1024])
                )
                emit_mul_send(b, i5, nsb, bc)
                if b + 1 < B:
                    if i5 == 0:
                        emit_xt_loads(b + 1, 0)
                        emit_xt_loads(b + 1, 1)
                    elif i5 in (2, 4):
                        emit_xt_loads(b + 1, i5 // 2 + 1)
                    # chain MMs only in the back half: during blocks 0-2 the
                    # previous batch's AllToAll transfer starves the DMA
                    # queues, and a stalled chain MM blocks the in-order PE
                    # queue (attention itself is SBUF-resident there).
                    for vj in {3: (0, 1), 4: (2, 3), 5: (4, 5), 6: (6, 7)}.get(i5, ()):
                        emit_v_chain(b + 1, vj)
                    if i5 in (3, 4):
                        emit_k_chain(b + 1, i5 - 3)
                    for qj in {5: (0, 1), 6: (2, 3), 7: (4, 5, 6, 7)}.get(i5, ()):
                        emit_q_chain(b + 1, qj)
                # previous batch: its AllToAll (issued at the end of batch
                # b-1) completes ~22us in; load its rv tiles at i5==2 and
                # run its out-proj during the back half of this batch
                if b > 0 and i5 == 4:
                    emit_rv_loads(b - 1)
                if b > 0 and i5 >= 5:
                    n_chunks = [3, 3, 2][i5 - 5]
                    base = [0, 3, 6][i5 - 5]
                    for ch in range(n_chunks):
                        emit_outproj_chunk(b - 1, base + ch)

            # ---- AllToAll for this batch: head-shard -> seq-shard
            nc.gpsimd.collective_compute(
                "AllToAll",
                Alu.bypass,
                replica_groups=[list(range(NCORES))],
                ins=[send[b][:, :, :].opt()],
                outs=[recv[b][:, :, :].opt()],
            )

        # keep the PE's HAM clock warm through the final AllToAll wait:
        # fp32 matmuls on resident data run at 1/4 rate (~850ns each), so a
        # dozen of them span ~10us of the ~16us window; without them the
        # out-proj drain starts at the cold 1.2 GHz clock.
        for w in range(12):
            warm = projp.tile([128, 512], F32, name=f"warm_{w}", tag="proj")
            nc.tensor.matmul(
                warm[:, :],
                lhsT=bob_sb[:, 0:128],
                rhs=bob_sb[:, 0:512],
                start=True, stop=True,
            )
        # drain the last batch's output projections, rotating over the
        # now-idle psum pools for pipelining depth
        emit_rv_loads(B - 1)
        for chunk in range(8):
            pool, tag = [(outpp, "ops"), (scp, "sc"),
                         (noutp, "nout"), (scp, "sc")][chunk % 4]
            emit_outproj_chunk(B - 1, chunk, pool=pool, tag=tag)


def _f8(a):
    return np.clip(a, -240.0, 240.0).astype(ml_dtypes.float8_e4m3)


def prep_in_maps(x, y, Wq, bq, Wk, bk, Wv, bv, Wo, bo):
    x = np.asarray(x, np.float32)
    y = np.asarray(y, np.float32)
    # paired fp8 layouts for DoubleRow: [.., kk, p, t, ..] with feature
    # index f = (2*kk + t)*128 + p
    xT = x.transpose(0, 2, 1)                       # [B, D, SQ]
    xt = _f8(np.ascontiguousarray(
        xT.reshape(B, KC2, 2, 128, SQ).transpose(0, 1, 3, 2, 4)))
    yT = y.transpose(0, 2, 1)                       # [B, DC, SK]
    yt = _f8(np.ascontiguousarray(
        yT.reshape(B, FC2, 2, 128, SK).transpose(0, 1, 3, 2, 4)))
    ytv = np.ascontiguousarray(
        yT.reshape(B, FC, 128, SK)).astype(ml_dtypes.bfloat16)
    bf = ml_dtypes.bfloat16
    wo = np.ascontiguousarray(
        np.asarray(Wo, np.float32).reshape(KC, 128, D)).astype(bf)
    bob = np.ascontiguousarray(
        np.broadcast_to(np.asarray(bo, np.float32)[None, :], (128, D))
    )
    in_maps = []
    for c in range(NCORES):
        cs = slice(c * 128, (c + 1) * 128)
        wq_c = np.asarray(Wq, np.float32)[:, cs] * 32.0
        wk_c = np.asarray(Wk, np.float32)[:, cs] * 32.0
        wv_c = np.asarray(Wv, np.float32)[:, cs]
        in_maps.append({
            "xt": xt,
            "yt": yt,
            "ytv": ytv,
            "wq": _f8(np.ascontiguousarray(
                wq_c.reshape(KC2, 2, 128, 128).transpose(0, 2, 1, 3))),
            "wk": _f8(np.ascontiguousarray(
                wk_c.reshape(FC2, 2, 128, 128).transpose(0, 2, 1, 3))),
            "wv": np.ascontiguousarray(wv_c.reshape(FC, 128, 128)).astype(bf),
            "wo": wo,
            "bq": np.ascontiguousarray(
                np.asarray(bq, np.float32)[cs].reshape(128, 1) * 32.0),
            "bk": np.ascontiguousarray(
                np.asarray(bk, np.float32)[cs].reshape(128, 1) * 32.0),
            "bvb": np.ascontiguousarray(
                np.broadcast_to(np.asarray(bv, np.float32)[cs][None, :], (128, 128))
            ),
            "bob": bob,
        })
    return in_maps


_NC_CACHE = None


def get_nc():
    global _NC_CACHE
    if _NC_CACHE is None:
        _NC_CACHE = build_nc()
    return _NC_CACHE


def run(in_maps, **kwargs):
    nc = get_nc()
    return bass_utils.run_bass_kernel_spmd(
        nc, in_maps, core_ids=list(range(NCORES)), **kwargs
    )


def gather(results):
    full = np.empty((B, SQ, D), np.float32)
    for c in range(NCORES):
        full[:, c * SQL:(c + 1) * SQL, :] = results[c]["out"]
    return full


def kernel(**inputs):
    in_maps = prep_in_maps(**inputs)
    res = run(in_maps)
    return gather(res.results)


if __name__ == "__main__":
    nc = build_nc()
    print("build OK")

